# revision 6
# baseline (speedup 1.0000x reference)
"""DGI (2-layer GCN + readout + bilinear discriminator) on 8 trn2 NeuronCores.

Strategy (graph-parallel, dst-owner edge sharding):
  - Nodes are sharded contiguously across the 8 cores (6250/core, padded to
    6272 = 49*128). Each core owns the edges whose dst falls in its shard
    (edges sorted by dst on host - pure index/layout preprocessing).
  - GCN aggregation is associativity-refactored:  (A_hat x) W  instead of
    A_hat (x W), and both the positive and negative (permuted) feature sets
    are aggregated together as one 2x-wide feature block per round.
  - Normalization is folded into the node tables:  with z' = dis * z,
       agg[n] = dis[n] * ( sum_{e: dst=n} z'[src_e]  +  z'[n] ).
    dis = (deg+1)^-1/2 is computed on-device by a one-hot matmul counting
    pass over the same chunk structure.
  - Per 128-edge chunk, messages are fetched with dma_gather (row gather
    from the replicated DRAM table) and aggregated into the 128-node PSUM
    tile with a one-hot selection matmul:  agg += S^T @ msgs, where
    S[e, n] = (dst_local[e] == n) is built on DVE with one is_equal.
  - The only collectives: AllGather of the (dis-scaled) layer-input tables
    (z1' : N x 256, h' : N x 512) and a 64x257 AllReduce for the readout.
  - dma_gather indices are int16, so tables are addressed as two halves
    (25088 rows each); chunks are half-pure by construction (host groups
    each tile's edges by source half; pads gather row 0 with a zeroed
    one-hot column).
"""

import numpy as np
from contextlib import ExitStack

import concourse.bass as bass
import concourse.bacc as bacc
import concourse.tile as tile
from concourse import bass_utils, mybir, library_config

F32 = mybir.dt.float32
I16 = mybir.dt.int16
I8 = mybir.dt.int8
AF = mybir.ActivationFunctionType
OP = mybir.AluOpType

NCORES = 8
NGRAPH = 64
D_IN = 128
D_H = 256


class _Cfg:
    def __init__(self, N, E, KA, KB):
        self.N = N
        self.E = E
        self.NPER = N // NCORES
        self.NTILES = (self.NPER + 127) // 128
        self.NPC = self.NTILES * 128           # padded nodes per core
        self.NPAD = self.NPC * NCORES          # padded table rows
        self.HALF = self.NPC * (NCORES // 2)   # table half boundary
        self.KA = KA                           # chunks per tile, half A
        self.KB = KB
        self.KT = KA + KB
        self.NCH = self.NTILES * self.KT       # chunks per core


def _host_prep(x, W1, b1, W2, b2, Wb, bb, src, dst, batch, perm):
    N, E = x.shape[0], src.shape[0]
    cfg0_NPER = N // NCORES
    NTILES = (cfg0_NPER + 127) // 128
    NPC = NTILES * 128
    HALF = NPC * (NCORES // 2)

    src = src.astype(np.int64)
    dst = dst.astype(np.int64)
    srcpad = (src // cfg0_NPER) * NPC + (src % cfg0_NPER)
    is_b = srcpad >= HALF

    # global tile id of each edge's dst
    core = dst // cfg0_NPER
    tloc = (dst % cfg0_NPER) // 128
    gtile = core * NTILES + tloc
    ngt = NCORES * NTILES

    nA = np.bincount(gtile[~is_b], minlength=ngt)
    nB = np.bincount(gtile[is_b], minlength=ngt)
    KA = max(1, int(-(-nA.max() // 128)))
    KB = max(1, int(-(-nB.max() // 128)))
    cfg = _Cfg(N, E, KA, KB)

    # per-(tile,half) slot assignment for every edge
    order = np.lexsort((is_b, gtile))          # sort by tile, A before B
    e_s = order                                 # edge ids in placement order
    g_s = gtile[order]
    b_s = is_b[order]
    # rank within (tile, half)
    grp = g_s * 2 + b_s
    changes = np.empty(E, np.bool_)
    changes[0] = True
    changes[1:] = grp[1:] != grp[:-1]
    starts = np.flatnonzero(changes)
    rank = np.arange(E) - np.repeat(starts, np.diff(np.append(starts, E)))

    # chunk (within core) and partition slot of each edge
    t_loc_s = g_s % NTILES
    core_s = g_s // NTILES
    chunk_in_half = rank // 128
    p_slot = rank % 128
    chunk = t_loc_s * cfg.KT + np.where(b_s, KA + chunk_in_half, chunk_in_half)

    dstloc_v = (dst[e_s] % cfg0_NPER) % 128    # dst slot within tile
    idx16_v = (srcpad[e_s] - np.where(b_s, HALF, 0)).astype(np.int16)

    # per-core arrays
    dstloc = np.full((NCORES, 128, cfg.NCH), -1.0, np.float32)
    gidx = np.zeros((NCORES, 16, 8 * cfg.NCH), np.int16)
    dstloc[core_s, p_slot, chunk] = dstloc_v.astype(np.float32)
    # gather position within the tile-half group: i = chunk_in_half*128 + p
    gi = chunk_in_half * 128 + p_slot
    # column base of the group (in 16-wide wrapped units of 8 cols per chunk)
    cbase = t_loc_s * cfg.KT + np.where(b_s, KA, 0)
    gidx[core_s, gi % 16, 8 * cbase + gi // 16] = idx16_v
    gidx = np.tile(gidx, (1, 8, 1))            # replicate to 128 partitions

    # xperm gather indices + mask, batch, x_own per core
    permN = perm.astype(np.int64)
    xpidxA = np.zeros((NCORES, 16, 8 * NTILES), np.int16)
    xpidxB = np.zeros((NCORES, 16, 8 * NTILES), np.int16)
    maskB = np.zeros((NCORES, 128, NTILES), np.int8)
    batchf = np.full((NCORES, 128, NTILES), -1.0, np.float32)
    x_own = np.zeros((NCORES, cfg.NPC, D_IN), np.float32)
    for c in range(NCORES):
        n0 = c * cfg0_NPER
        nv = cfg0_NPER
        x_own[c, :nv] = x[n0 : n0 + nv]
        pv = permN[n0 : n0 + nv]
        pb = pv >= HALF
        ia = np.where(pb, 0, pv).astype(np.int16)
        ib = np.where(pb, pv - HALF, 0).astype(np.int16)
        pos = np.arange(nv)
        t_of = pos // 128
        i_of = pos % 128
        xpidxA[c, i_of % 16, 8 * t_of + i_of // 16] = ia
        xpidxB[c, i_of % 16, 8 * t_of + i_of // 16] = ib
        maskB[c, i_of, t_of] = pb.astype(np.int8)
        batchf[c, i_of, t_of] = batch[n0 : n0 + nv].astype(np.float32)
    xpidxA = np.tile(xpidxA, (1, 8, 1))
    xpidxB = np.tile(xpidxB, (1, 8, 1))

    # shared constant layouts
    W2r = np.ascontiguousarray(
        W2.reshape(2, 128, D_H).transpose(1, 0, 2).reshape(128, 2 * D_H)
    ).astype(np.float32)
    WbT = np.ascontiguousarray(Wb.T)
    WbTr = np.ascontiguousarray(
        WbT.reshape(2, 128, D_H).transpose(1, 0, 2).reshape(128, 2 * D_H)
    ).astype(np.float32)
    shared = dict(
        x=np.ascontiguousarray(x, np.float32),
        W1=np.ascontiguousarray(W1, np.float32),
        W2r=W2r,
        WbTr=WbTr,
        b1b=np.tile(np.asarray(b1, np.float32)[None, :], (128, 1)),
        b2b=np.tile(np.asarray(b2, np.float32)[None, :], (128, 1)),
        bbb=np.full((128, 1), float(bb), np.float32),
        iota=np.ascontiguousarray(
            np.broadcast_to(np.arange(128, dtype=np.float32), (128, 128))
        ),
        ident=np.eye(128, dtype=np.float32),
        ones=np.ones((128, 1), np.float32),
    )
    in_maps = []
    for c in range(NCORES):
        m = dict(shared)
        m.update(
            x_own=np.ascontiguousarray(x_own[c]),
            gidx=np.ascontiguousarray(gidx[c]),
            dstloc=np.ascontiguousarray(dstloc[c]),
            xpidxA=np.ascontiguousarray(xpidxA[c]),
            xpidxB=np.ascontiguousarray(xpidxB[c]),
            maskB=np.ascontiguousarray(maskB[c]),
            batchf=np.ascontiguousarray(batchf[c]),
        )
        in_maps.append(m)
    return cfg, in_maps


def _build(cfg):
    c = cfg
    nc = bacc.Bacc("TRN2", target_bir_lowering=False, debug=False,
                   enable_asserts=False, num_devices=NCORES)

    # I/O
    x = nc.dram_tensor("x", [c.N, D_IN], F32, kind="ExternalInput")
    W1 = nc.dram_tensor("W1", [D_IN, D_H], F32, kind="ExternalInput")
    W2r = nc.dram_tensor("W2r", [128, 2 * D_H], F32, kind="ExternalInput")
    WbTr = nc.dram_tensor("WbTr", [128, 2 * D_H], F32, kind="ExternalInput")
    b1b = nc.dram_tensor("b1b", [128, D_H], F32, kind="ExternalInput")
    b2b = nc.dram_tensor("b2b", [128, D_H], F32, kind="ExternalInput")
    bbb = nc.dram_tensor("bbb", [128, 1], F32, kind="ExternalInput")
    iota = nc.dram_tensor("iota", [128, 128], F32, kind="ExternalInput")
    ident = nc.dram_tensor("ident", [128, 128], F32, kind="ExternalInput")
    ones = nc.dram_tensor("ones", [128, 1], F32, kind="ExternalInput")
    x_own = nc.dram_tensor("x_own", [c.NPC, D_IN], F32, kind="ExternalInput")
    gidx = nc.dram_tensor("gidx", [128, 8 * c.NCH], I16, kind="ExternalInput")
    dstloc = nc.dram_tensor("dstloc", [128, c.NCH], F32, kind="ExternalInput")
    xpidxA = nc.dram_tensor("xpidxA", [128, 8 * c.NTILES], I16, kind="ExternalInput")
    xpidxB = nc.dram_tensor("xpidxB", [128, 8 * c.NTILES], I16, kind="ExternalInput")
    maskB = nc.dram_tensor("maskB", [128, c.NTILES], I8, kind="ExternalInput")
    batchf = nc.dram_tensor("batchf", [128, c.NTILES], F32, kind="ExternalInput")
    h1s = nc.dram_tensor("h1s", [c.NPC, D_H], F32, kind="ExternalOutput")
    sc1s = nc.dram_tensor("sc1s", [c.NPC], F32, kind="ExternalOutput")
    sc2s = nc.dram_tensor("sc2s", [c.NPC], F32, kind="ExternalOutput")

    XH = c.HALF if c.HALF < c.N else c.N // 2  # x-table half boundary

    with tile.TileContext(nc) as tc, ExitStack() as ctx:
        nc.gpsimd.load_library(library_config.mlp)

        cp = ctx.enter_context(tc.tile_pool(name="const", bufs=1))
        wp = ctx.enter_context(tc.tile_pool(name="work", bufs=3))
        sp = ctx.enter_context(tc.tile_pool(name="sel", bufs=4))
        mp = ctx.enter_context(tc.tile_pool(name="msgs", bufs=4))
        ip = ctx.enter_context(tc.tile_pool(name="idx", bufs=4))
        pa = ctx.enter_context(tc.tile_pool(name="pa", bufs=2, space="PSUM"))
        pt = ctx.enter_context(tc.tile_pool(name="pt", bufs=2, space="PSUM"))
        ph = ctx.enter_context(tc.tile_pool(name="ph", bufs=2, space="PSUM"))
        rp = ctx.enter_context(tc.tile_pool(name="ro", bufs=1, space="PSUM"))
        dp = ctx.enter_context(tc.tile_pool(name="dram", bufs=1, space="DRAM"))

        def load_const(t, src_ap):
            tl = cp.tile(list(src_ap.shape), src_ap.dtype, tag=t)
            nc.sync.dma_start(tl[:], src_ap)
            return tl

        W1_r = load_const("W1", W1[:, :])
        W2_r = load_const("W2", W2r[:, :])
        WbT_r = load_const("WbT", WbTr[:, :])
        b1_r = load_const("b1", b1b[:, :])
        b2_r = load_const("b2", b2b[:, :])
        bb_r = load_const("bb", bbb[:, :])
        iota_r = load_const("iota", iota[:, :])
        id_r = load_const("id", ident[:, :])
        on_r = load_const("ones", ones[:, :])
        dl_r = load_const("dstloc", dstloc[:, :])
        mk_r = load_const("maskB", maskB[:, :])
        bf_r = load_const("batchf", batchf[:, :])
        dis_r = cp.tile([128, c.NTILES], F32, tag="dis")

        zsh = dp.tile([c.NPC, 2 * D_IN], F32)
        z1tab = dp.tile([c.NPAD, 2 * D_IN], F32, addr_space="Shared")
        hsh = dp.tile([c.NPC, 2 * D_H], F32)
        htab = dp.tile([c.NPAD, 2 * D_H], F32, addr_space="Shared")
        h1scr = dp.tile([c.NPC, D_H], F32)
        h2scr = dp.tile([c.NPC, D_H], F32)
        ro_in = dp.tile([64, D_H + 1], F32)
        ro_out = dp.tile([64, D_H + 1], F32, addr_space="Shared")

        def mk_S(cc):
            S = sp.tile([128, 128], F32, tag="S")
            nc.vector.tensor_tensor(
                out=S[:], in0=iota_r[:],
                in1=dl_r[:, cc : cc + 1].to_broadcast([128, 128]),
                op=OP.is_equal)
            return S

        # ---- P1: degree -> dis = 1/sqrt(deg+1) --------------------------
        for t in range(c.NTILES):
            dg = pt.tile([128, 1], F32, tag="tp")
            for j in range(c.KT):
                S = mk_S(t * c.KT + j)
                nc.tensor.matmul(out=dg[:], lhsT=S[:], rhs=on_r[:],
                                 start=(j == 0), stop=(j == c.KT - 1))
            sq = wp.tile([128, 1], F32, tag="sq")
            nc.scalar.activation(sq[:], dg[:], AF.Sqrt, bias=1.0)
            nc.vector.reciprocal(dis_r[:, t : t + 1], sq[:])

        # ---- P3: build z1' shard = dis * [x_own, x[perm]] ---------------
        for t in range(c.NTILES):
            rows = slice(t * 128, (t + 1) * 128)
            xo = wp.tile([128, D_IN], F32, tag="xo")
            nc.sync.dma_start(xo[:], x_own[rows, :])
            ia = ip.tile([128, 8], I16, tag="gi")
            nc.sync.dma_start(ia[:], xpidxA[:, 8 * t : 8 * (t + 1)])
            xpA = wp.tile([128, 1, D_IN], F32, tag="xpA")
            nc.gpsimd.dma_gather(
                out_ap=xpA[:], in_ap=x[0:XH, :], idxs_ap=ia[:],
                num_idxs=128, num_idxs_reg=128, elem_size=D_IN)
            ib = ip.tile([128, 8], I16, tag="gi")
            nc.sync.dma_start(ib[:], xpidxB[:, 8 * t : 8 * (t + 1)])
            xpB = wp.tile([128, 1, D_IN], F32, tag="xpB")
            nc.gpsimd.dma_gather(
                out_ap=xpB[:], in_ap=x[XH : c.N, :], idxs_ap=ib[:],
                num_idxs=128, num_idxs_reg=128, elem_size=D_IN)
            xp = wp.tile([128, D_IN], F32, tag="xp")
            nc.vector.tensor_copy(xp[:], xpA[:, 0, :])
            nc.vector.copy_predicated(
                xp[:], mk_r[:, t : t + 1].to_broadcast([128, D_IN]), xpB[:, 0, :])
            za = wp.tile([128, D_IN], F32, tag="za")
            nc.vector.tensor_scalar(za[:], xo[:], dis_r[:, t : t + 1], None, OP.mult)
            zb = wp.tile([128, D_IN], F32, tag="zb")
            nc.vector.tensor_scalar(zb[:], xp[:], dis_r[:, t : t + 1], None, OP.mult)
            nc.sync.dma_start(zsh[rows, 0:D_IN], za[:])
            nc.sync.dma_start(zsh[rows, D_IN : 2 * D_IN], zb[:])

        nc.gpsimd.collective_compute(
            "AllGather", OP.bypass, replica_groups=[list(range(NCORES))],
            ins=[zsh.opt()], outs=[z1tab.opt()])

        # ---- shared aggregation loop ------------------------------------
        GMAX = 8

        def agg_round(rnd, tabA, tabB, D, epilogue):
            for t in range(c.NTILES):
                views = []
                for tab, cstart, K in ((tabA, 0, c.KA), (tabB, c.KA, c.KB)):
                    done = 0
                    while done < K:
                        g = min(GMAX, K - done)
                        c0 = t * c.KT + cstart + done
                        gi = ip.tile([128, 8 * g], I16, tag="gi")
                        nc.sync.dma_start(gi[:], gidx[:, 8 * c0 : 8 * (c0 + g)])
                        mt = mp.tile([128, g, D], F32, tag="m")
                        nc.gpsimd.dma_gather(
                            out_ap=mt[:], in_ap=tab, idxs_ap=gi[:],
                            num_idxs=128 * g, num_idxs_reg=128 * g, elem_size=D)
                        views.extend(mt[:, j, :] for j in range(g))
                        done += g
                agg = pa.tile([128, D], F32, tag="agg")
                for j in range(c.KT):
                    S = mk_S(t * c.KT + j)
                    nc.tensor.matmul(out=agg[:], lhsT=S[:], rhs=views[j],
                                     start=(j == 0), stop=(j == c.KT - 1))
                epilogue(t, agg)

        # ---- P5: round 1 + layer-1 --------------------------------------
        def epi1(t, agg):
            rows = slice(t * 128, (t + 1) * 128)
            own = wp.tile([128, 2 * D_IN], F32, tag="own1")
            nc.sync.dma_start(own[:], zsh[rows, :])
            t1 = wp.tile([128, 2 * D_IN], F32, tag="t11")
            nc.vector.tensor_tensor(out=t1[:], in0=agg[:], in1=own[:], op=OP.add)
            a1 = wp.tile([128, 2 * D_IN], F32, tag="a11")
            nc.vector.tensor_scalar(a1[:], t1[:], dis_r[:, t : t + 1], None, OP.mult)
            for h in (0, 1):
                tp = pt.tile([128, 128], F32, tag="tp")
                nc.tensor.transpose(tp[:], a1[:, h * 128 : (h + 1) * 128], id_r[:])
                tps = wp.tile([128, 128], F32, tag="tps")
                nc.scalar.copy(tps[:], tp[:])
                hp = ph.tile([128, D_H], F32, tag="hps")
                nc.tensor.matmul(out=hp[:], lhsT=tps[:], rhs=W1_r[:],
                                 start=True, stop=True)
                hb = wp.tile([128, D_H], F32, tag="hb")
                nc.vector.tensor_tensor(out=hb[:], in0=hp[:], in1=b1_r[:], op=OP.add)
                hr = wp.tile([128, D_H], F32, tag="hr")
                nc.vector.tensor_scalar(hr[:], hb[:], 0.0, None, OP.max)
                hs = wp.tile([128, D_H], F32, tag="hs")
                nc.vector.tensor_scalar(hs[:], hr[:], dis_r[:, t : t + 1], None, OP.mult)
                nc.sync.dma_start(hsh[rows, h * D_H : (h + 1) * D_H], hs[:])

        agg_round(1, z1tab[0 : c.HALF, :], z1tab[c.HALF : c.NPAD, :],
                  2 * D_IN, epi1)

        nc.gpsimd.collective_compute(
            "AllGather", OP.bypass, replica_groups=[list(range(NCORES))],
            ins=[hsh.opt()], outs=[htab.opt()])

        # ---- P7: round 2 + layer-2 + readout partials -------------------
        ro = rp.tile([64, D_H], F32)
        cnt = rp.tile([64, 1], F32)

        def epi2(t, agg):
            rows = slice(t * 128, (t + 1) * 128)
            own = wp.tile([128, 2 * D_H], F32, tag="own2")
            nc.sync.dma_start(own[:], hsh[rows, :])
            t1 = wp.tile([128, 2 * D_H], F32, tag="t12")
            nc.vector.tensor_tensor(out=t1[:], in0=agg[:], in1=own[:], op=OP.add)
            a2 = wp.tile([128, 2 * D_H], F32, tag="a12")
            nc.vector.tensor_scalar(a2[:], t1[:], dis_r[:, t : t + 1], None, OP.mult)
            for h in (0, 1):
                hp = ph.tile([128, D_H], F32, tag="hps")
                for k in (0, 1):
                    tp = pt.tile([128, 128], F32, tag="tp")
                    nc.tensor.transpose(
                        tp[:], a2[:, h * D_H + k * 128 : h * D_H + (k + 1) * 128],
                        id_r[:])
                    tps = wp.tile([128, 128], F32, tag="tps")
                    nc.scalar.copy(tps[:], tp[:])
                    nc.tensor.matmul(
                        out=hp[:], lhsT=tps[:], rhs=W2_r[:, k * D_H : (k + 1) * D_H],
                        start=(k == 0), stop=(k == 1))
                ho = wp.tile([128, D_H], F32, tag="ho")
                nc.vector.tensor_tensor(out=ho[:], in0=hp[:], in1=b2_r[:], op=OP.add)
                if h == 0:
                    nc.sync.dma_start(h1s[rows, :], ho[:])
                    nc.sync.dma_start(h1scr[rows, :], ho[:])
                    B = sp.tile([128, 64], F32, tag="B")
                    nc.vector.tensor_tensor(
                        out=B[:], in0=iota_r[:, 0:64],
                        in1=bf_r[:, t : t + 1].to_broadcast([128, 64]),
                        op=OP.is_equal)
                    nc.tensor.matmul(out=ro[:], lhsT=B[:], rhs=ho[:],
                                     start=(t == 0), stop=(t == c.NTILES - 1))
                    nc.tensor.matmul(out=cnt[:], lhsT=B[:], rhs=on_r[:],
                                     start=(t == 0), stop=(t == c.NTILES - 1))
                else:
                    nc.sync.dma_start(h2scr[rows, :], ho[:])

        agg_round(2, htab[0 : c.HALF, :], htab[c.HALF : c.NPAD, :],
                  2 * D_H, epi2)

        # ---- P8: readout AllReduce -> c -> cW ---------------------------
        ro_sb = wp.tile([64, D_H], F32, tag="rosb")
        nc.vector.tensor_copy(ro_sb[:], ro[:])
        cnt_sb = wp.tile([64, 1], F32, tag="cntsb")
        nc.vector.tensor_copy(cnt_sb[:], cnt[:])
        nc.sync.dma_start(ro_in[:, 0:D_H], ro_sb[:])
        nc.sync.dma_start(ro_in[:, D_H : D_H + 1], cnt_sb[:])
        nc.gpsimd.collective_compute(
            "AllReduce", OP.add, replica_groups=[list(range(NCORES))],
            ins=[ro_in.opt()], outs=[ro_out.opt()])
        rr = wp.tile([64, D_H + 1], F32, tag="rr")
        nc.sync.dma_start(rr[:], ro_out[:, :])
        c1 = wp.tile([64, 1], F32, tag="c1")
        nc.vector.tensor_scalar(c1[:], rr[:, D_H : D_H + 1], 1.0, None, OP.max)
        rec = wp.tile([64, 1], F32, tag="rec")
        nc.vector.reciprocal(rec[:], c1[:])
        mean = wp.tile([64, D_H], F32, tag="mean")
        nc.vector.tensor_scalar(mean[:], rr[:, 0:D_H], rec[:], None, OP.mult)
        csb = wp.tile([64, D_H], F32, tag="csb")
        nc.scalar.activation(csb[:], mean[:], AF.Sigmoid)
        cw = ph.tile([64, D_H], F32, tag="hps")
        for k in (0, 1):
            ct = pt.tile([128, 64], F32, tag="tp")
            nc.tensor.transpose(ct[:], csb[:, k * 128 : (k + 1) * 128],
                                id_r[0:64, 0:64])
            cts = wp.tile([128, 64], F32, tag="cts")
            nc.scalar.copy(cts[:], ct[:])
            nc.tensor.matmul(out=cw[:], lhsT=cts[:],
                             rhs=WbT_r[:, k * D_H : (k + 1) * D_H],
                             start=(k == 0), stop=(k == 1))
        cw_sb = wp.tile([64, D_H], F32, tag="cwsb")
        nc.vector.tensor_copy(cw_sb[:], cw[:])

        # ---- P9: scores --------------------------------------------------
        for t in range(c.NTILES):
            rows = slice(t * 128, (t + 1) * 128)
            B = sp.tile([128, 64], F32, tag="B")
            nc.vector.tensor_tensor(
                out=B[:], in0=iota_r[:, 0:64],
                in1=bf_r[:, t : t + 1].to_broadcast([128, 64]), op=OP.is_equal)
            bt = pt.tile([64, 128], F32, tag="tp")
            nc.tensor.transpose(bt[:], B[:], id_r[:])
            bts = wp.tile([64, 128], F32, tag="bts")
            nc.scalar.copy(bts[:], bt[:])
            tmm = ph.tile([128, D_H], F32, tag="hps")
            nc.tensor.matmul(out=tmm[:], lhsT=bts[:], rhs=cw_sb[:],
                             start=True, stop=True)
            for which, scr, out_sc in ((0, h1scr, sc1s), (1, h2scr, sc2s)):
                ht = wp.tile([128, D_H], F32, tag="ht")
                nc.sync.dma_start(ht[:], scr[rows, :])
                pr = wp.tile([128, D_H], F32, tag="pr")
                nc.vector.tensor_tensor(out=pr[:], in0=ht[:], in1=tmm[:], op=OP.mult)
                sc = wp.tile([128, 1], F32, tag="sc")
                nc.vector.reduce_sum(out=sc[:], in_=pr[:], axis=mybir.AxisListType.X)
                scb = wp.tile([128, 1], F32, tag="scb")
                nc.vector.tensor_tensor(out=scb[:], in0=sc[:], in1=bb_r[:], op=OP.add)
                nc.sync.dma_start(out_sc[rows], scb[:])

    nc.compile()
    return nc


_CACHE = {}


def _get_program(cfg):
    key = (cfg.N, cfg.E, cfg.KA, cfg.KB)
    if key not in _CACHE:
        _CACHE[key] = _build(cfg)
    return _CACHE[key]


def kernel(x, W1, b1, W2, b2, Wb, bb, src, dst, batch, perm):
    x = np.asarray(x, np.float32)
    cfg, in_maps = _host_prep(
        x, np.asarray(W1), np.asarray(b1), np.asarray(W2), np.asarray(b2),
        np.asarray(Wb), np.asarray(bb), np.asarray(src), np.asarray(dst),
        np.asarray(batch), np.asarray(perm))
    nc = _get_program(cfg)
    res = bass_utils.run_bass_kernel_spmd(nc, in_maps, core_ids=list(range(NCORES)))
    N, NPER = cfg.N, cfg.NPER
    logits = np.empty(2 * N, np.float32)
    h1 = np.empty((N, D_H), np.float32)
    for c in range(NCORES):
        r = res.results[c]
        h1[c * NPER : (c + 1) * NPER] = r["h1s"][:NPER]
        logits[c * NPER : (c + 1) * NPER] = r["sc1s"][:NPER]
        logits[N + c * NPER : N + (c + 1) * NPER] = r["sc2s"][:NPER]
    return logits, h1
